# revision 32
# baseline (speedup 1.0000x reference)
"""LoRALinear kernel for Trainium2 (8 NeuronCores, SPMD data-parallel).

Computes out = x @ W.T + b + SCALE*((x@gA.T)@gB.T + (x@lA.T)@lB.T)
  x: [8, 2048, 1024] f32, W: [4096, 1024], b: [4096]
  gA/lA: [8, 1024], gB/lB: [4096, 8]  ->  out: [8, 2048, 4096] f32

Strategy: the rank-16 LoRA update is folded into the weights on the
host (O(r*d_in*d_out) = 0.05% of total FLOPs) and both GEMM operands
are marshaled to the [partition, k-tile, col] layout the PE array
needs (contraction dim on partitions), fp16:
  WeT3[p, k, o] = W_eff.T[k*128+p, o],  W_eff = W + SCALE*(gB@gA+lB@lA)
  xT3[p, k, s]  = x[i].T[k*128+p, s]   per core

Device (per core, one batch of x): a pure dense GEMM at the fp16
roofline (216 ns per 128x128x512 matmul at 2.4 GHz).  Both operands
stay resident in SBUF (96 KiB/partition).  TRN2 has two HWDGE rings,
each FIFO with ~0.6us fixed cost per dma_start, so chunks are issued
in exactly compute-consumption order: the sync ring carries the first
x chunk, then W o-chunks interleaved with bias o-chunks (pacing the
ot loop), then the late x s-quarters; the scalar ring carries the
early x chunks and then all 128 output stores so stores never queue
behind input loads.  The s-range is processed in 4 quarters with
o-tiles outer within a quarter, so the first psum group needs only
~1.5 MB of operands before the matmul stream starts; after that the
stream runs gap-free to the end (the first ~8 matmuls ride the HAM
clock ramp, ~2us, cheaper than any warmup scheme that queues ahead of
real work in the PE's FIFO).  The bias arrives host-pre-broadcast
[128, DOUT] fp16 and is added during psum eviction by a mixed-dtype
DVE tensor_tensor (f32 psum + f16 bias -> f32 SBUF), then DMA out.

fp16 operand rounding gives ~2.4e-4 absmax relative error vs the f32
reference; accumulation stays f32 in PSUM.
"""
import numpy as np
from contextlib import ExitStack

import concourse.bass as bass
import concourse.tile as tile
from concourse import bacc, mybir
from concourse.bass import ts, ds
from concourse.bass_utils import run_bass_kernel_spmd

F32 = mybir.dt.float32
F16 = mybir.dt.float16

N_CORES = 8
B, S, DIN, DOUT, R = 8, 2048, 1024, 4096, 8
SCALE = 16.0 / 8

P = 128            # partition tile
OTILE = 512        # matmul moving free dim (one PSUM bank of f32)
KT = DIN // P      # 8 k-tiles
OT = DOUT // OTILE # 8 o-tiles
ST = S // P        # 16 s-tiles
SQ = 4             # s-quarters (4 s-tiles each)
STQ = ST // SQ
SQW = STQ * P      # columns of x per s-quarter
HALF = OTILE // 2  # first-pass split of the W o-chunk


def build_nc():
    nc = bacc.Bacc("TRN2", target_bir_lowering=False, debug=False,
                   num_devices=N_CORES)
    xT3 = nc.dram_tensor("xT3", [P, KT, S], F16, kind="ExternalInput").ap()
    WeT3 = nc.dram_tensor("WeT3", [P, KT, DOUT], F16, kind="ExternalInput").ap()
    bias_bc = nc.dram_tensor("bias_bc", [P, DOUT], F16, kind="ExternalInput").ap()
    out = nc.dram_tensor("out", [S, DOUT], F32, kind="ExternalOutput").ap()

    with tile.TileContext(nc) as tc:
        with ExitStack() as ctx:
            const = ctx.enter_context(tc.tile_pool(name="const", bufs=1))
            xw_pool = ctx.enter_context(tc.tile_pool(name="xw", bufs=1))
            out_pool = ctx.enter_context(tc.tile_pool(name="outp", bufs=4))
            ps512 = ctx.enter_context(tc.tile_pool(name="ps512", bufs=7, space="PSUM"))

            # ---- resident operands: x.T and W_eff.T, fp16, 3D tiles ----
            xsb = xw_pool.tile([P, KT, S], F16, name="xsb")
            wet = xw_pool.tile([P, KT, DOUT], F16, name="wet")
            bias_sb = const.tile([P, DOUT], F16)

            # Two HWDGE rings (each FIFO, ~0.6us fixed/DMA), both issued in
            # exactly consumption order:
            #  - sync ring: W o-chunks interleaved with bias o-chunks
            #    (paces the ot loop), then the x s-quarter bulk
            #  - scalar ring: the small early x chunks (first psum groups),
            #    then all output stores
            nc.sync.dma_start(xsb[:, :, 0:2 * P], xT3[:, :, 0:2 * P])
            for j in range(OT):
                nc.sync.dma_start(wet[:, :, ts(j, OTILE)], WeT3[:, :, ts(j, OTILE)])
                nc.sync.dma_start(bias_sb[:, ts(j, OTILE)], bias_bc[:, ts(j, OTILE)])
            for q in range(2, SQ):
                nc.sync.dma_start(xsb[:, :, ts(q, SQW)], xT3[:, :, ts(q, SQW)])
            nc.scalar.dma_start(xsb[:, :, 2 * P:SQW], xT3[:, :, 2 * P:SQW])
            nc.scalar.dma_start(xsb[:, :, ts(1, SQW)], xT3[:, :, ts(1, SQW)])

            # ---- main GEMM: out[s, o] = x @ W_effT + bias ----
            for sq in range(SQ):
                for ot in range(OT):
                    for stq in range(STQ):
                        st = sq * STQ + stq
                        po = ps512.tile([P, OTILE], F32, tag="ps512")
                        for kt in range(KT):
                            nc.tensor.matmul(po[:], xsb[:, kt, ts(st, P)],
                                             wet[:, kt, ts(ot, OTILE)],
                                             start=(kt == 0), stop=(kt == KT - 1))
                        osb = out_pool.tile([P, OTILE], F32)
                        nc.vector.tensor_tensor(osb[:], po[:],
                                                bias_sb[:, ts(ot, OTILE)],
                                                mybir.AluOpType.add)
                        # output stores on the second HWDGE ring (scalar);
                        # the final group alternates rings so the last two
                        # stores' fixed costs overlap instead of serializing
                        last_grp = (sq == SQ - 1 and ot == OT - 1)
                        eng = nc.sync if (last_grp and stq % 2 == 0) else nc.scalar
                        eng.dma_start(out[ts(st, P), ts(ot, OTILE)], osb[:])

    nc.compile()
    return nc


_NC_CACHE = None


def _get_nc():
    global _NC_CACHE
    if _NC_CACHE is None:
        _NC_CACHE = build_nc()
    return _NC_CACHE


def make_in_maps(x, W, b, global_A, global_B, local_A, local_B):
    x = np.asarray(x, dtype=np.float32)
    W = np.asarray(W, dtype=np.float32)
    bias_bc = np.ascontiguousarray(np.broadcast_to(
        np.asarray(b, dtype=np.float32).astype(np.float16), (P, DOUT)))
    lora = (np.asarray(global_B, dtype=np.float32) @ np.asarray(global_A, dtype=np.float32)
            + np.asarray(local_B, dtype=np.float32) @ np.asarray(local_A, dtype=np.float32))
    W_eff16 = (W + SCALE * lora).astype(np.float16)        # [DOUT, DIN]
    # WeT3[p, k, o] = W_eff[o, k*128+p]
    WeT3 = np.ascontiguousarray(
        W_eff16.reshape(DOUT, KT, P).transpose(2, 1, 0))
    x16 = x.astype(np.float16)                             # [B, S, DIN]
    return [
        # xT3[p, k, s] = x[i][s, k*128+p]
        {"xT3": np.ascontiguousarray(x16[i].reshape(S, KT, P).transpose(2, 1, 0)),
         "WeT3": WeT3, "bias_bc": bias_bc}
        for i in range(N_CORES)
    ]


def kernel(x, W, b, global_A, global_B, local_A, local_B):
    nc = _get_nc()
    in_maps = make_in_maps(x, W, b, global_A, global_B, local_A, local_B)
    res = run_bass_kernel_spmd(nc, in_maps, list(range(N_CORES))).results
    return np.stack([res[i]["out"] for i in range(N_CORES)], axis=0)
